# revision 125
# baseline (speedup 1.0000x reference)
"""AttnBlock (LayerNorm + single-head self-attention + proj + residual) on 8
Trainium2 NeuronCores.

Problem: x [4, 512, 64, 64] f32; per batch image: t = LN(x) over channels;
qkv = t @ w_qkv.T; attn = softmax(q k^T / sqrt(c)); out = attn v @ w_proj.T;
y = x + out.

Sharding: 8 cores = 4 batches x 2 query-halves. Each core gets its batch's
full image (token order rolled so its 2048 queries are local tokens 0..2047),
computes LN + K/V over all 4096 tokens and Q over its half, then
scores/softmax/attn-V/proj for its 2048 queries. No collectives.

v2 design notes (vs the bf16 baseline):
  - All QKV projections run as fp8e4m3 DoubleRow matmuls (contraction 256/pass)
    with host-prepared channel-paired weights scaled by 8 (keeps the N(0,1/512)
    weights out of the subnormal range). The x8 factors cancel in the exp scale
    (SCALE/64) and the final 1/(8*den) normalizer.
  - LayerNorm applied in bf16 on DVE (2x mode); rstd computed as
    exp(-0.5*ln(var+eps)) on ACT (no DVE reciprocal); gamma applied via
    per-partition scalar in the STT; beta added by the fp8 cast op.
  - Softmax denominator accumulated on the (otherwise idle) DVE as fp8-input
    pair adds, partition-reduced by two ones-column matmuls at qb end so its
    PSUM bank stays free for the next qb's tail.
  - DMAs are batched 3D transfers (~30 total vs 144); the per-dma_start issue
    cost on gpsimd is ~1us regardless of size. Q tiles stay resident in SBUF.
  - PSUM evictions are pair-batched ([128,2,512] = 2 banks) and spread across
    ACT/DVE/GPSIMD to balance engine load.
"""
import numpy as np

import concourse.bass as bass
import concourse.tile as tile
from concourse import mybir
from concourse.bass_utils import run_bass_kernel_spmd

P = 128
C = 512          # channels
T = 4096         # tokens per image
TQ = 2048        # queries per core
CB = C // P      # 4 channel chunks
W2 = CB // 2     # 2 channel chunk-pairs
TBLK = 512       # token block for LN/QKV phase
NTB = T // TBLK  # 8
NQB = TQ // TBLK  # 4 query blocks
NKT = T // P     # 32 key chunks
F32 = mybir.dt.float32
BF16 = mybir.dt.bfloat16
FP8 = mybir.dt.float8e4
FP = mybir.ActivationFunctionType
ALU = mybir.AluOpType
DR = mybir.MatmulPerfMode.DoubleRow
SCALE = float(C) ** -0.5
WS = 8.0          # fp8 weight prescale


def split_multiwaits(nc, max_waits=1):
    """walrus codegen allows one sync-wait slot on most TPB instruction
    structs; Tile's sem assignment emits several. Split extras into
    wait-only EventSemaphore instructions on the same engine stream."""
    n = 0
    for fn in nc.m.functions:
        for blk in fn.blocks:
            out = []
            for inst in blk.instructions:
                si = inst.sync_info
                if si is not None and si.on_wait is not None and len(si.on_wait) > max_waits:
                    extra = list(si.on_wait[:-max_waits])
                    keep = list(si.on_wait[-max_waits:])
                    for w in extra:
                        ev = mybir.InstEventSemaphore(
                            name=nc.get_next_instruction_name(),
                            engine=inst.engine,
                            sync_info=mybir.SyncInfo(on_wait=[w], on_update=[]),
                        )
                        out.append(ev)
                        n += 1
                    si.on_wait = keep
                out.append(inst)
            blk.instructions[:] = out
    return n


def build_nc(plain_gb=False):
    """plain_gb: gamma==1 and beta==0 (host-verified) — the LN-apply multiply
    writes fp8 directly, skipping the per-chunk gamma/beta ACT cast."""
    nc = bass.Bass()
    xbf = nc.declare_dram_parameter("xbf", [C, T], BF16, isOutput=False)
    xres = nc.declare_dram_parameter("xres", [TQ, C], F32, isOutput=False)
    wq8d = nc.declare_dram_parameter("wq8d", [W2, P, 2, 3 * C], FP8, isOutput=False)
    wp8d = nc.declare_dram_parameter("wp8d", [W2, P, 2, C], FP8, isOutput=False)
    gamma = nc.declare_dram_parameter("gamma", [C], F32, isOutput=False)
    beta = nc.declare_dram_parameter("beta", [C], F32, isOutput=False)
    out = nc.declare_dram_parameter("out", [TQ, C], F32, isOutput=True)
    rec_dram = nc.dram_tensor("rec_dram", [NQB, TBLK], F32)

    with tile.TileContext(nc) as tc:
        with (
            tc.tile_pool(name="xs", bufs=3) as xs,
            tc.tile_pool(name="consts", bufs=1) as consts,
            tc.tile_pool(name="resid", bufs=1) as resid,
        ):
            # prefetch x block 0 + weights before anything else (per-chunk DMAs
            # so the first stats matmul can start after the first chunk lands)
            xb0 = consts.tile([P, CB, TBLK], BF16, tag="xb0", name="xb0")
            for cc in range(CB):
                nc.gpsimd.dma_start(
                    out=xb0[:, cc, :], in_=xbf[cc * P:(cc + 1) * P, 0:TBLK])
            # block 1's x must beat the weight/const DMAs into the issue queue
            # (b1(1) runs ~10us in; 8 queued const issues would starve it)
            xb1_pre = xs.tile([P, CB, TBLK], BF16, tag="xb", name="xb1_pre")
            nc.gpsimd.dma_start(
                out=xb1_pre,
                in_=xbf[:, TBLK:2 * TBLK].rearrange("(cb p) t -> p cb t", cb=CB))
            wq8 = []
            for w in range(W2):
                t = consts.tile([P, 2, 3 * C], FP8, tag=f"wq8_{w}", name=f"wq8_{w}")
                nc.gpsimd.dma_start(out=t, in_=wq8d[w])
                wq8.append(t)
            # ---- constants ----
            gcolt = consts.tile([P, CB], F32, tag="gcolt")
            nc.gpsimd.dma_start(out=gcolt, in_=gamma.rearrange("(cb p) -> p cb", cb=CB))
            bcolt = consts.tile([P, CB], F32, tag="bcolt")
            nc.gpsimd.dma_start(out=bcolt, in_=beta.rearrange("(cb p) -> p cb", cb=CB))
            wp8 = []
            for w in range(W2):
                t = consts.tile([P, 2, C], FP8, tag=f"wp8_{w}", name=f"wp8_{w}")
                nc.gpsimd.dma_start(out=t, in_=wp8d[w])
                wp8.append(t)
            ones_col_bf = consts.tile([P, 1], BF16, tag="ones_col_bf")
            nc.vector.memset(ones_col_bf, 1.0)
            ones_row = consts.tile([1, P], BF16, tag="ones_row")
            nc.vector.memset(ones_row, 1.0)
            # den stationary: [128,2,1] slice of a wide tile (dual-fp8 LDWEIGHTS
            # rejects a free-standing [128,2,1] with tiny row stride)
            ones_pair_w = consts.tile([P, 2, TBLK], FP8, tag="ones_pair_w")
            nc.vector.memset(ones_pair_w, 1.0)
            ones_pair8 = ones_pair_w[:, :, 0:1]
            eps_t = consts.tile([1, 1], F32, tag="eps_t")
            nc.vector.memset(eps_t, 1e-5)
            ones11 = consts.tile([1, 1], F32, tag="ones11")
            nc.vector.memset(ones11, 1.0)
            neg2 = consts.tile([P, 1], F32, tag="neg2")
            nc.vector.memset(neg2, -2.0)

            # ---- resident tensors ----
            KT = []   # K^T pairs: 2 x [128, 2, 4096] fp8 (DoubleRow layout)
            for w in range(W2):
                KT.append(resid.tile([P, 2, T], FP8, tag=f"KT{w}", name=f"KT{w}"))
            V = []    # V [tokenpair, d]: 16 x [128, 2, 512] fp8 (DoubleRow layout)
            for u in range(NKT // 2):
                V.append(resid.tile([P, 2, C], FP8, tag=f"V{u}", name=f"V{u}"))
            QT = []   # resident Q: per qb, per w: [128, 2, 512] fp8
            for qb in range(NQB):
                QT.append([resid.tile([P, 2, TBLK], FP8, tag=f"QT{qb}_{w}",
                                      name=f"QT{qb}_{w}") for w in range(W2)])

            # =========== Phase B: LN + QKV ===========
            with (
                tc.tile_pool(name="sqs", bufs=2) as sqs,
                tc.tile_pool(name="stat", bufs=1) as stat,
                tc.tile_pool(name="rows", bufs=2) as rows,
                tc.tile_pool(name="bcs", bufs=2) as bcs,
                tc.tile_pool(name="lns", bufs=3) as lns,
                tc.tile_pool(name="ztmp", bufs=3) as ztmp,
                tc.tile_pool(name="ps_row", bufs=1, space="PSUM") as ps_row,
                tc.tile_pool(name="ps_q", bufs=1, space="PSUM") as ps_q,
            ):
                muneg_bf = [None] * NTB
                rstd_bf = [None] * NTB
                xb_t = [None] * NTB
                q_slot = [0]

                def pair_psum(prefix, tb):
                    tag = f"pq{q_slot[0] % 3}"
                    q_slot[0] += 1
                    return ps_q.tile([P, 2, TBLK], F32, tag=tag,
                                     name=f"{prefix}{tb}")

                def load_xb(tb):
                    if tb >= NTB or xb_t[tb] is not None:
                        return
                    if tb == 0:
                        xb_t[tb] = xb0
                        return
                    if tb == 1:
                        xb_t[tb] = xb1_pre
                        return
                    ts = slice(tb * TBLK, (tb + 1) * TBLK)
                    xb = xs.tile([P, CB, TBLK], BF16, tag="xb", name=f"xb{tb}")
                    nc.gpsimd.dma_start(
                        out=xb, in_=xbf[:, ts].rearrange("(cb p) t -> p cb t", cb=CB))
                    xb_t[tb] = xb

                def b1_block(tb):
                    load_xb(tb)
                    load_xb(tb + 1)  # prefetch next block's x a step early
                    xb = xb_t[tb]
                    sq = sqs.tile([P, CB, TBLK], BF16, tag="sq", name=f"sq{tb}")
                    nc.vector.tensor_mul(out=sq, in0=xb, in1=xb)
                    s1 = ps_row.tile([1, TBLK], F32, tag="s1", name=f"s1_{tb}")
                    for cc in range(CB):
                        nc.tensor.matmul(s1, ones_col_bf, xb[:, cc, :],
                                         start=(cc == 0), stop=(cc == CB - 1))
                    s2 = ps_row.tile([1, TBLK], F32, tag="s2", name=f"s2_{tb}")
                    for cc in range(CB):
                        nc.tensor.matmul(s2, ones_col_bf, sq[:, cc, :],
                                         start=(cc == 0), stop=(cc == CB - 1))
                    # -mu (bf16, for the broadcast + x-mu subtract-as-add)
                    mnb = stat.tile([1, TBLK], BF16, tag=f"mneg{tb}", name=f"mneg{tb}")
                    nc.scalar.activation(out=mnb, in_=s1, func=FP.Copy, scale=-1.0 / C)
                    muneg_bf[tb] = mnb
                    musq = rows.tile([1, TBLK], F32, tag="musq", name=f"musq{tb}")
                    nc.gpsimd.tensor_mul(out=musq, in0=mnb, in1=mnb)
                    var = rows.tile([1, TBLK], F32, tag="var", name=f"var{tb}")
                    nc.vector.scalar_tensor_tensor(
                        out=var, in0=s2, scalar=1.0 / C, in1=musq,
                        op0=ALU.mult, op1=ALU.subtract)
                    lnv = rows.tile([1, TBLK], F32, tag="lnv", name=f"lnv{tb}")
                    nc.scalar.activation(out=lnv, in_=var, func=FP.Ln, bias=eps_t)
                    rb = stat.tile([1, TBLK], BF16, tag=f"rstd{tb}", name=f"rstd{tb}")
                    nc.scalar.activation(out=rb, in_=lnv, func=FP.Exp, scale=-0.5)
                    rstd_bf[tb] = rb

                # ---- BZ: LN apply -> fp8 lnp tiles (runs a step ahead of B2
                # so the serial bc->za->zb->z8 chain never stalls the PE) ----
                lnp_t = [None] * NTB
                bc_t = [None] * NTB

                def z_bc(tb):
                    # broadcast -mu/rstd rows -> [P, 2, TBLK] bf16 (emitted
                    # before b1(t) so the cast clears the DVE queue early)
                    bc_ps = pair_psum("bc", tb)
                    nc.tensor.matmul(bc_ps[:, 0, :], ones_row, muneg_bf[tb],
                                     start=True, stop=True)
                    nc.tensor.matmul(bc_ps[:, 1, :], ones_row, rstd_bf[tb],
                                     start=True, stop=True)
                    bc = bcs.tile([P, 2, TBLK], BF16, tag="bc", name=f"bc{tb}")
                    nc.vector.tensor_copy(out=bc, in_=bc_ps)
                    bc_t[tb] = bc

                def z_block(tb):
                    xb = xb_t[tb]
                    bc = bc_t[tb]
                    # LN apply: za = x - mu, zb = za*rstd (bf16 TT, 2x DVE);
                    # t8 = fp8(zb*gamma + beta) fused into the ACT cast.
                    lnp = []
                    for w in range(W2):
                        lnp.append(lns.tile([P, 2, TBLK], FP8, tag=f"lnp{w}",
                                            name=f"lnp{tb}_{w}"))
                    for cc in range(CB):
                        za = ztmp.tile([P, TBLK], BF16, tag="za", name=f"za{tb}_{cc}")
                        nc.vector.tensor_add(out=za, in0=xb[:, cc, :], in1=bc[:, 0, :])
                        dst = lnp[cc // 2][:, cc % 2, :]
                        if plain_gb:
                            nc.vector.tensor_mul(out=dst, in0=za, in1=bc[:, 1, :])
                        else:
                            zb = ztmp.tile([P, TBLK], BF16, tag="zb",
                                           name=f"zb{tb}_{cc}")
                            nc.vector.tensor_mul(out=zb, in0=za, in1=bc[:, 1, :])
                            nc.scalar.activation(out=dst, in_=zb, func=FP.Identity,
                                                 scale=gcolt[:, cc:cc + 1],
                                                 bias=bcolt[:, cc:cc + 1])
                    lnp_t[tb] = lnp

                # ---- B2: QKV (fp8 DoubleRow) ----
                def b2_block(tb):
                    ts = slice(tb * TBLK, (tb + 1) * TBLK)
                    lnp = lnp_t[tb]
                    # K^T: 2 psum pairs, each fed by 2 DR matmuls
                    for w in range(W2):
                        kp = pair_psum(f"kp{w}_", tb)
                        for j in range(2):
                            dd = 2 * w + j
                            for v in range(W2):
                                nc.tensor.matmul(
                                    kp[:, j, :],
                                    wq8[v][:, :, C + dd * P:C + (dd + 1) * P],
                                    lnp[v], perf_mode=DR,
                                    start=(v == 0), stop=(v == W2 - 1))
                        nc.scalar.activation(out=KT[w][:, :, ts], in_=kp, func=FP.Copy)
                    # V: 2 psum pairs (token-chunk pairs) -> resident V tiles
                    for m in range(2):
                        vp = pair_psum(f"vp{m}_", tb)
                        for j in range(2):
                            tt = 2 * m + j
                            for v in range(W2):
                                nc.tensor.matmul(
                                    vp[:, j, :],
                                    lnp[v][:, :, tt * P:(tt + 1) * P],
                                    wq8[v][:, :, 2 * C:3 * C], perf_mode=DR,
                                    start=(v == 0), stop=(v == W2 - 1))
                        if (plain_gb and m == 0) or tb == NTB - 1:
                            # last block: DVE gates the B->C handoff while ACT
                            # has slack there
                            nc.scalar.activation(out=V[tb * 2 + m], in_=vp,
                                                 func=FP.Copy)
                        else:
                            nc.vector.tensor_copy(out=V[tb * 2 + m], in_=vp)
                    # Q^T (local queries only) -> resident QT tiles
                    if tb < NQB:
                        for w in range(W2):
                            qp = pair_psum(f"qp{w}_", tb)
                            for j in range(2):
                                dd = 2 * w + j
                                for v in range(W2):
                                    nc.tensor.matmul(
                                        qp[:, j, :],
                                        wq8[v][:, :, dd * P:(dd + 1) * P],
                                        lnp[v], perf_mode=DR,
                                        start=(v == 0), stop=(v == W2 - 1))
                            if tb == NQB - 1:
                                # ACT gates the B->C handoff; DVE has slack
                                nc.vector.tensor_copy(out=QT[tb][w], in_=qp)
                            else:
                                nc.scalar.activation(out=QT[tb][w], in_=qp,
                                                     func=FP.Copy)

                for step in range(NTB + 2):
                    if step < NTB:
                        b1_block(step)
                    if 1 <= step < NTB + 1:
                        z_bc(step - 1)
                        z_block(step - 1)
                    if step >= 2:
                        b2_block(step - 2)

            # =========== Phase C: attention ===========
            with (
                tc.tile_pool(name="es", bufs=6) as es,
                tc.tile_pool(name="outts", bufs=2) as outts,
                tc.tile_pool(name="dens", bufs=2) as dens,
                tc.tile_pool(name="fins", bufs=2) as fins,
                tc.tile_pool(name="xrs", bufs=2) as xrs,
                tc.tile_pool(name="ps_s", bufs=3, space="PSUM") as ps_s,
                tc.tile_pool(name="ps_o", bufs=1, space="PSUM") as ps_o,
                tc.tile_pool(name="ps_d", bufs=1, space="PSUM") as ps_d,
            ):
                def make_tail(qb, outTp, den_ps, xr, last=False):
                    def tail():
                        qs = slice(qb * TBLK, (qb + 1) * TBLK)
                        den_row = dens.tile([1, TBLK], F32, tag="den_row",
                                            name=f"den_row{qb}")
                        nc.vector.tensor_copy(out=den_row, in_=den_ps)
                        # [1,512] -> [128,4] partition-major via 4 PE transposes
                        # (a DRAM roundtrip here costs ~5us of dead latency on
                        # the final tail)
                        den_pm = ps_d.tile([P, CB], F32, tag="pd",
                                           name=f"den_pm{qb}")
                        for qq in range(CB):
                            nc.tensor.transpose(den_pm[:, qq:qq + 1],
                                                den_row[:, qq * P:(qq + 1) * P],
                                                ones11)
                        recT = dens.tile([P, CB], F32, tag="recT", name=f"recT{qb}")
                        nc.vector.reciprocal(out=recT, in_=den_pm)
                        finb = fins.tile([P, CB, C], F32, tag="finb", name=f"finb{qb}")
                        # proj (fp8 DR) + normalize + residual per 128-query slice
                        for qq in range(CB):
                            # the last tail has no kt-loop to overlap with; the
                            # pscr banks are free then, so rotate pf through
                            # them to break the pf->fin->pf serial chain
                            if last:
                                pf = ps_s.tile([P, C], F32, tag="pscr",
                                               name=f"pf{qb}_{qq}")
                            else:
                                pf = ps_d.tile([P, C], F32, tag="pd",
                                               name=f"pf{qb}_{qq}")
                            for w in range(W2):
                                nc.tensor.matmul(
                                    pf, outTp[w][:, :, qq * P:(qq + 1) * P],
                                    wp8[w], perf_mode=DR,
                                    start=(w == 0), stop=(w == W2 - 1))
                            nc.vector.scalar_tensor_tensor(
                                out=finb[:, qq, :], in0=pf,
                                scalar=recT[:, qq:qq + 1], in1=xr[:, qq, :],
                                op0=ALU.mult, op1=ALU.add)
                            rsl = slice(qb * TBLK + qq * P, qb * TBLK + (qq + 1) * P)
                            nc.gpsimd.dma_start(out=out[rsl, :], in_=finb[:, qq, :])
                    return tail

                pending_tail = None
                for qb in range(NQB):
                    qs = slice(qb * TBLK, (qb + 1) * TBLK)
                    # prefetch residual rows for this qb's tail
                    xr = xrs.tile([P, CB, C], F32, tag="xr", name=f"xr{qb}")
                    nc.gpsimd.dma_start(
                        out=xr,
                        in_=xres[qs, :].rearrange("(qq p) c -> p qq c", qq=CB))
                    pop = [ps_o.tile([P, 2, TBLK], F32, tag=f"po{w}",
                                     name=f"po{qb}_{w}") for w in range(W2)]
                    dacc = dens.tile([P, 2, TBLK], F32, tag="dacc", name=f"dacc{qb}")
                    dacc_bf = dens.tile([P, 2, TBLK], BF16, tag="dacc_bf",
                                        name=f"dacc_bf{qb}")

                    pair_t = {}

                    def scores_exp(kt, qb=qb):
                        u = kt // 2
                        if kt % 2 == 0:
                            pair_t[u] = es.tile([P, 2, TBLK], FP8, tag="e",
                                                name=f"e{qb}_{u}")
                        ksl = slice(kt * P, (kt + 1) * P)
                        pscr = ps_s.tile([P, TBLK], F32, tag="pscr",
                                         name=f"pscr{qb}_{kt}")
                        for w in range(W2):
                            nc.tensor.matmul(pscr, KT[w][:, :, ksl], QT[qb][w],
                                             perf_mode=DR,
                                             start=(w == 0), stop=(w == W2 - 1))
                        # shifted exp (softmax-invariant) keeps E in fp8e4m3 range
                        nc.scalar.activation(out=pair_t[u][:, kt % 2, :], in_=pscr,
                                             func=FP.Exp, scale=SCALE / (WS * WS),
                                             bias=neg2)

                    scores_exp(0)
                    scores_exp(1)
                    for kt in range(NKT):
                        u = kt // 2
                        if kt + 2 < NKT:
                            scores_exp(kt + 2)
                        if kt % 2 == 1:
                            for cc in range(CB):
                                nc.tensor.matmul(
                                    pop[cc // 2][:, cc % 2, :],
                                    V[u][:, :, cc * P:(cc + 1) * P], pair_t[u],
                                    perf_mode=DR,
                                    start=(u == 0), stop=(u == NKT // 2 - 1))
                            # denominator partial sums on the (idle) DVE; the
                            # last pair skips the accumulator (it feeds the
                            # PSUM reduce directly via a DR ones-matmul) so the
                            # qb-end chain doesn't wait an add+cast after the
                            # final exp
                            if u == 0:
                                nc.vector.tensor_copy(out=dacc, in_=pair_t[u])
                            elif u < NKT // 2 - 1:
                                nc.vector.tensor_add(out=dacc, in0=dacc,
                                                     in1=pair_t[u])
                                if u == NKT // 2 - 2:
                                    nc.vector.tensor_copy(out=dacc_bf, in_=dacc)
                        if kt == 6 and pending_tail is not None:
                            pending_tail()
                            pending_tail = None
                    # partition-reduce: dacc_bf (u0..14, cast early) + the last
                    # exp pair directly (den's PSUM bank stays free until here
                    # so the next qb's tail, which shares it, never waits)
                    den_ps = ps_d.tile([1, TBLK], F32, tag="pd", name=f"den{qb}")
                    for j in range(2):
                        nc.tensor.matmul(den_ps, ones_col_bf, dacc_bf[:, j, :],
                                         start=(j == 0), stop=False)
                    nc.tensor.matmul(den_ps, ones_pair8, pair_t[NKT // 2 - 1],
                                     perf_mode=DR, start=False, stop=True)
                    # evict numerators to fp8 (scaled by 1/WS^2: pf comes out as
                    # num*wp, normalized by 1/den at the fin STT)
                    outTp = []
                    for w in range(W2):
                        t = outts.tile([P, 2, TBLK], FP8, tag=f"outT{w}",
                                       name=f"outT{qb}_{w}")
                        if w == 0:
                            nc.scalar.activation(out=t, in_=pop[w], func=FP.Copy,
                                                 scale=1.0 / (WS * WS))
                        else:
                            # DVE is idle at qb wind-down now that the dacc
                            # chain ends at u==14; ACT is the backlogged engine
                            nc.vector.tensor_scalar_mul(t, pop[w], 1.0 / (WS * WS))
                        outTp.append(t)
                    pending_tail = make_tail(qb, outTp, den_ps, xr,
                                             last=(qb == NQB - 1))
                if pending_tail is not None:
                    pending_tail()
    split_multiwaits(nc)
    return nc


_NC = {}


def kernel(x, ln_gamma, ln_beta, w_qkv, w_proj, **run_kwargs):
    import ml_dtypes
    x = np.ascontiguousarray(np.asarray(x, dtype=np.float32))
    ln_gamma = np.asarray(ln_gamma, dtype=np.float32)
    ln_beta = np.asarray(ln_beta, dtype=np.float32)
    fp8_np = mybir.dt.np(FP8)
    # channel-paired fp8 qkv weights, prescaled by WS:
    # wq8[w, p, j, d] = w_qkv[d, w*256 + j*128 + p] * WS
    wq8 = np.ascontiguousarray(
        (np.asarray(w_qkv, dtype=np.float32).T * WS)
        .reshape(W2, 2, P, 3 * C).transpose(0, 2, 1, 3)).astype(fp8_np)
    wp8 = np.ascontiguousarray(
        (np.asarray(w_proj, dtype=np.float32).T * WS)
        .reshape(W2, 2, P, C).transpose(0, 2, 1, 3)).astype(fp8_np)
    b, c, h, w = x.shape
    assert (b, c, h * w) == (4, C, T)

    in_maps = []
    for core in range(8):
        bi, half = core // 2, core % 2
        xt_b = x[bi].reshape(C, T)
        if half == 0:
            xt_i = xt_b
        else:
            xt_i = np.concatenate([xt_b[:, TQ:], xt_b[:, :TQ]], axis=1)
        xt_i = np.ascontiguousarray(xt_i)
        xres_i = np.ascontiguousarray(xt_i[:, :TQ].T)
        in_maps.append({
            "xbf": xt_i.astype(ml_dtypes.bfloat16),
            "xres": xres_i, "wq8d": wq8, "wp8d": wp8,
            "gamma": ln_gamma, "beta": ln_beta,
        })

    # specialized plain-gamma/beta build measured slower (DVE became the
    # phase-B cap once the fp8 LN writes moved there); keep the general path
    plain_gb = False
    if plain_gb not in _NC:
        _NC[plain_gb] = build_nc(plain_gb=plain_gb)
    res = run_bass_kernel_spmd(_NC[plain_gb], in_maps, core_ids=list(range(8)),
                               **run_kwargs)

    y = np.empty((b, T, C), dtype=np.float32)
    for core in range(8):
        bi, half = core // 2, core % 2
        y[bi, half * TQ:(half + 1) * TQ, :] = res.results[core]["out"]
    y = np.ascontiguousarray(y.transpose(0, 2, 1).reshape(b, C, h, w))
    if run_kwargs:
        return y, res
    return y


# revision 126
# speedup vs baseline: 1.0370x; 1.0370x over previous
"""AttnBlock (LayerNorm + single-head self-attention + proj + residual) on 8
Trainium2 NeuronCores.

Problem: x [4, 512, 64, 64] f32; per batch image: t = LN(x) over channels;
qkv = t @ w_qkv.T; attn = softmax(q k^T / sqrt(c)); out = attn v @ w_proj.T;
y = x + out.

Sharding: 8 cores = 4 batches x 2 query-halves. Each core gets its batch's
full image (token order rolled so its 2048 queries are local tokens 0..2047),
computes LN + K/V over all 4096 tokens and Q over its half, then
scores/softmax/attn-V/proj for its 2048 queries. No collectives.

v2 design notes (vs the bf16 baseline):
  - All QKV projections run as fp8e4m3 DoubleRow matmuls (contraction 256/pass)
    with host-prepared channel-paired weights scaled by 8 (keeps the N(0,1/512)
    weights out of the subnormal range). The x8 factors cancel in the exp scale
    (SCALE/64) and the final 1/(8*den) normalizer.
  - LayerNorm applied in bf16 on DVE (2x mode); rstd computed as
    exp(-0.5*ln(var+eps)) on ACT (no DVE reciprocal); gamma applied via
    per-partition scalar in the STT; beta added by the fp8 cast op.
  - Softmax denominator accumulated on the (otherwise idle) DVE as fp8-input
    pair adds, partition-reduced by two ones-column matmuls at qb end so its
    PSUM bank stays free for the next qb's tail.
  - DMAs are batched 3D transfers (~30 total vs 144); the per-dma_start issue
    cost on gpsimd is ~1us regardless of size. Q tiles stay resident in SBUF.
  - PSUM evictions are pair-batched ([128,2,512] = 2 banks) and spread across
    ACT/DVE/GPSIMD to balance engine load.
"""
import numpy as np

import concourse.bass as bass
import concourse.tile as tile
from concourse import mybir
from concourse.bass_utils import run_bass_kernel_spmd

P = 128
C = 512          # channels
T = 4096         # tokens per image
TQ = 2048        # queries per core
CB = C // P      # 4 channel chunks
W2 = CB // 2     # 2 channel chunk-pairs
TBLK = 512       # token block for LN/QKV phase
NTB = T // TBLK  # 8
NQB = TQ // TBLK  # 4 query blocks
NKT = T // P     # 32 key chunks
F32 = mybir.dt.float32
BF16 = mybir.dt.bfloat16
FP8 = mybir.dt.float8e4
FP = mybir.ActivationFunctionType
ALU = mybir.AluOpType
DR = mybir.MatmulPerfMode.DoubleRow
SCALE = float(C) ** -0.5
WS = 8.0          # fp8 weight prescale


def split_multiwaits(nc, max_waits=1):
    """walrus codegen allows one sync-wait slot on most TPB instruction
    structs; Tile's sem assignment emits several. Split extras into
    wait-only EventSemaphore instructions on the same engine stream."""
    n = 0
    for fn in nc.m.functions:
        for blk in fn.blocks:
            out = []
            for inst in blk.instructions:
                si = inst.sync_info
                if si is not None and si.on_wait is not None and len(si.on_wait) > max_waits:
                    extra = list(si.on_wait[:-max_waits])
                    keep = list(si.on_wait[-max_waits:])
                    for w in extra:
                        ev = mybir.InstEventSemaphore(
                            name=nc.get_next_instruction_name(),
                            engine=inst.engine,
                            sync_info=mybir.SyncInfo(on_wait=[w], on_update=[]),
                        )
                        out.append(ev)
                        n += 1
                    si.on_wait = keep
                out.append(inst)
            blk.instructions[:] = out
    return n


def build_nc(plain_gb=False):
    """plain_gb: gamma==1 and beta==0 (host-verified) — the LN-apply multiply
    writes fp8 directly, skipping the per-chunk gamma/beta ACT cast."""
    nc = bass.Bass()
    xbf = nc.declare_dram_parameter("xbf", [C, T], BF16, isOutput=False)
    xres = nc.declare_dram_parameter("xres", [TQ, C], F32, isOutput=False)
    wq8d = nc.declare_dram_parameter("wq8d", [W2, P, 2, 3 * C], FP8, isOutput=False)
    wp8d = nc.declare_dram_parameter("wp8d", [W2, P, 2, C], FP8, isOutput=False)
    gamma = nc.declare_dram_parameter("gamma", [C], F32, isOutput=False)
    beta = nc.declare_dram_parameter("beta", [C], F32, isOutput=False)
    out = nc.declare_dram_parameter("out", [TQ, C], F32, isOutput=True)
    rec_dram = nc.dram_tensor("rec_dram", [NQB, TBLK], F32)

    with tile.TileContext(nc) as tc:
        with (
            tc.tile_pool(name="xs", bufs=3) as xs,
            tc.tile_pool(name="consts", bufs=1) as consts,
            tc.tile_pool(name="resid", bufs=1) as resid,
        ):
            # prefetch x block 0 + weights before anything else (per-chunk DMAs
            # so the first stats matmul can start after the first chunk lands)
            xb0 = consts.tile([P, CB, TBLK], BF16, tag="xb0", name="xb0")
            for cc in range(CB):
                nc.gpsimd.dma_start(
                    out=xb0[:, cc, :], in_=xbf[cc * P:(cc + 1) * P, 0:TBLK])
            # block 1's x must beat the weight/const DMAs into the issue queue
            # (b1(1) runs ~10us in; 8 queued const issues would starve it)
            xb1_pre = xs.tile([P, CB, TBLK], BF16, tag="xb", name="xb1_pre")
            nc.gpsimd.dma_start(
                out=xb1_pre,
                in_=xbf[:, TBLK:2 * TBLK].rearrange("(cb p) t -> p cb t", cb=CB))
            wq8 = []
            for w in range(W2):
                t = consts.tile([P, 2, 3 * C], FP8, tag=f"wq8_{w}", name=f"wq8_{w}")
                nc.gpsimd.dma_start(out=t, in_=wq8d[w])
                wq8.append(t)
            # ---- constants ----
            gcolt = consts.tile([P, CB], F32, tag="gcolt")
            nc.gpsimd.dma_start(out=gcolt, in_=gamma.rearrange("(cb p) -> p cb", cb=CB))
            bcolt = consts.tile([P, CB], F32, tag="bcolt")
            nc.gpsimd.dma_start(out=bcolt, in_=beta.rearrange("(cb p) -> p cb", cb=CB))
            wp8 = []
            for w in range(W2):
                t = consts.tile([P, 2, C], FP8, tag=f"wp8_{w}", name=f"wp8_{w}")
                nc.gpsimd.dma_start(out=t, in_=wp8d[w])
                wp8.append(t)
            ones_col_bf = consts.tile([P, 1], BF16, tag="ones_col_bf")
            nc.vector.memset(ones_col_bf, 1.0)
            ones_row = consts.tile([1, P], BF16, tag="ones_row")
            nc.vector.memset(ones_row, 1.0)
            # den stationary: [128,2,1] slice of a wide tile (dual-fp8 LDWEIGHTS
            # rejects a free-standing [128,2,1] with tiny row stride)
            ones_pair_w = consts.tile([P, 2, TBLK], FP8, tag="ones_pair_w")
            nc.vector.memset(ones_pair_w, 1.0)
            ones_pair8 = ones_pair_w[:, :, 0:1]
            eps_t = consts.tile([1, 1], F32, tag="eps_t")
            nc.vector.memset(eps_t, 1e-5)
            ones11 = consts.tile([1, 1], F32, tag="ones11")
            nc.vector.memset(ones11, 1.0)
            neg2 = consts.tile([P, 1], F32, tag="neg2")
            nc.vector.memset(neg2, -2.0)

            # ---- resident tensors ----
            KT = []   # K^T pairs: 2 x [128, 2, 4096] fp8 (DoubleRow layout)
            for w in range(W2):
                KT.append(resid.tile([P, 2, T], FP8, tag=f"KT{w}", name=f"KT{w}"))
            V = []    # V [tokenpair, d]: 16 x [128, 2, 512] fp8 (DoubleRow layout)
            for u in range(NKT // 2):
                V.append(resid.tile([P, 2, C], FP8, tag=f"V{u}", name=f"V{u}"))
            QT = []   # resident Q: per qb, per w: [128, 2, 512] fp8
            for qb in range(NQB):
                QT.append([resid.tile([P, 2, TBLK], FP8, tag=f"QT{qb}_{w}",
                                      name=f"QT{qb}_{w}") for w in range(W2)])

            # =========== Phase B: LN + QKV ===========
            with (
                tc.tile_pool(name="sqs", bufs=2) as sqs,
                tc.tile_pool(name="stat", bufs=1) as stat,
                tc.tile_pool(name="rows", bufs=2) as rows,
                tc.tile_pool(name="bcs", bufs=2) as bcs,
                tc.tile_pool(name="lns", bufs=3) as lns,
                tc.tile_pool(name="ztmp", bufs=3) as ztmp,
                tc.tile_pool(name="ps_row", bufs=1, space="PSUM") as ps_row,
                tc.tile_pool(name="ps_q", bufs=1, space="PSUM") as ps_q,
            ):
                muneg_bf = [None] * NTB
                rstd_bf = [None] * NTB
                xb_t = [None] * NTB
                q_slot = [0]

                def pair_psum(prefix, tb):
                    tag = f"pq{q_slot[0] % 3}"
                    q_slot[0] += 1
                    return ps_q.tile([P, 2, TBLK], F32, tag=tag,
                                     name=f"{prefix}{tb}")

                def load_xb(tb):
                    if tb >= NTB or xb_t[tb] is not None:
                        return
                    if tb == 0:
                        xb_t[tb] = xb0
                        return
                    if tb == 1:
                        xb_t[tb] = xb1_pre
                        return
                    ts = slice(tb * TBLK, (tb + 1) * TBLK)
                    xb = xs.tile([P, CB, TBLK], BF16, tag="xb", name=f"xb{tb}")
                    nc.gpsimd.dma_start(
                        out=xb, in_=xbf[:, ts].rearrange("(cb p) t -> p cb t", cb=CB))
                    xb_t[tb] = xb

                def b1_block(tb):
                    load_xb(tb)
                    load_xb(tb + 1)  # prefetch next block's x a step early
                    xb = xb_t[tb]
                    sq = sqs.tile([P, CB, TBLK], BF16, tag="sq", name=f"sq{tb}")
                    nc.vector.tensor_mul(out=sq, in0=xb, in1=xb)
                    s1 = ps_row.tile([1, TBLK], F32, tag="s1", name=f"s1_{tb}")
                    for cc in range(CB):
                        nc.tensor.matmul(s1, ones_col_bf, xb[:, cc, :],
                                         start=(cc == 0), stop=(cc == CB - 1))
                    s2 = ps_row.tile([1, TBLK], F32, tag="s2", name=f"s2_{tb}")
                    for cc in range(CB):
                        nc.tensor.matmul(s2, ones_col_bf, sq[:, cc, :],
                                         start=(cc == 0), stop=(cc == CB - 1))
                    # -mu (bf16, for the broadcast + x-mu subtract-as-add)
                    mnb = stat.tile([1, TBLK], BF16, tag=f"mneg{tb}", name=f"mneg{tb}")
                    nc.scalar.activation(out=mnb, in_=s1, func=FP.Copy, scale=-1.0 / C)
                    muneg_bf[tb] = mnb
                    musq = rows.tile([1, TBLK], F32, tag="musq", name=f"musq{tb}")
                    nc.gpsimd.tensor_mul(out=musq, in0=mnb, in1=mnb)
                    var = rows.tile([1, TBLK], F32, tag="var", name=f"var{tb}")
                    nc.vector.scalar_tensor_tensor(
                        out=var, in0=s2, scalar=1.0 / C, in1=musq,
                        op0=ALU.mult, op1=ALU.subtract)
                    lnv = rows.tile([1, TBLK], F32, tag="lnv", name=f"lnv{tb}")
                    nc.scalar.activation(out=lnv, in_=var, func=FP.Ln, bias=eps_t)
                    rb = stat.tile([1, TBLK], BF16, tag=f"rstd{tb}", name=f"rstd{tb}")
                    nc.scalar.activation(out=rb, in_=lnv, func=FP.Exp, scale=-0.5)
                    rstd_bf[tb] = rb

                # ---- BZ: LN apply -> fp8 lnp tiles (runs a step ahead of B2
                # so the serial bc->za->zb->z8 chain never stalls the PE) ----
                lnp_t = [None] * NTB
                bc_t = [None] * NTB

                def z_bc(tb):
                    # broadcast -mu/rstd rows -> [P, 2, TBLK] bf16 (emitted
                    # before b1(t) so the cast clears the DVE queue early)
                    bc_ps = pair_psum("bc", tb)
                    nc.tensor.matmul(bc_ps[:, 0, :], ones_row, muneg_bf[tb],
                                     start=True, stop=True)
                    nc.tensor.matmul(bc_ps[:, 1, :], ones_row, rstd_bf[tb],
                                     start=True, stop=True)
                    bc = bcs.tile([P, 2, TBLK], BF16, tag="bc", name=f"bc{tb}")
                    nc.vector.tensor_copy(out=bc, in_=bc_ps)
                    bc_t[tb] = bc

                def z_block(tb):
                    xb = xb_t[tb]
                    bc = bc_t[tb]
                    # LN apply: za = x - mu, zb = za*rstd (bf16 TT, 2x DVE);
                    # t8 = fp8(zb*gamma + beta) fused into the ACT cast.
                    lnp = []
                    for w in range(W2):
                        lnp.append(lns.tile([P, 2, TBLK], FP8, tag=f"lnp{w}",
                                            name=f"lnp{tb}_{w}"))
                    for cc in range(CB):
                        za = ztmp.tile([P, TBLK], BF16, tag="za", name=f"za{tb}_{cc}")
                        nc.vector.tensor_add(out=za, in0=xb[:, cc, :], in1=bc[:, 0, :])
                        dst = lnp[cc // 2][:, cc % 2, :]
                        if plain_gb:
                            nc.vector.tensor_mul(out=dst, in0=za, in1=bc[:, 1, :])
                        else:
                            zb = ztmp.tile([P, TBLK], BF16, tag="zb",
                                           name=f"zb{tb}_{cc}")
                            nc.vector.tensor_mul(out=zb, in0=za, in1=bc[:, 1, :])
                            nc.scalar.activation(out=dst, in_=zb, func=FP.Identity,
                                                 scale=gcolt[:, cc:cc + 1],
                                                 bias=bcolt[:, cc:cc + 1])
                    lnp_t[tb] = lnp

                # ---- B2: QKV (fp8 DoubleRow) ----
                def b2_block(tb):
                    ts = slice(tb * TBLK, (tb + 1) * TBLK)
                    lnp = lnp_t[tb]
                    # K^T: 2 psum pairs, each fed by 2 DR matmuls
                    for w in range(W2):
                        kp = pair_psum(f"kp{w}_", tb)
                        for j in range(2):
                            dd = 2 * w + j
                            for v in range(W2):
                                nc.tensor.matmul(
                                    kp[:, j, :],
                                    wq8[v][:, :, C + dd * P:C + (dd + 1) * P],
                                    lnp[v], perf_mode=DR,
                                    start=(v == 0), stop=(v == W2 - 1))
                        nc.scalar.activation(out=KT[w][:, :, ts], in_=kp, func=FP.Copy)
                    # V: 2 psum pairs (token-chunk pairs) -> resident V tiles
                    for m in range(2):
                        vp = pair_psum(f"vp{m}_", tb)
                        for j in range(2):
                            tt = 2 * m + j
                            for v in range(W2):
                                nc.tensor.matmul(
                                    vp[:, j, :],
                                    lnp[v][:, :, tt * P:(tt + 1) * P],
                                    wq8[v][:, :, 2 * C:3 * C], perf_mode=DR,
                                    start=(v == 0), stop=(v == W2 - 1))
                        if (plain_gb and m == 0) or tb == NTB - 1:
                            # last block: DVE gates the B->C handoff while ACT
                            # has slack there
                            nc.scalar.activation(out=V[tb * 2 + m], in_=vp,
                                                 func=FP.Copy)
                        else:
                            nc.vector.tensor_copy(out=V[tb * 2 + m], in_=vp)
                    # Q^T (local queries only) -> resident QT tiles
                    if tb < NQB:
                        for w in range(W2):
                            qp = pair_psum(f"qp{w}_", tb)
                            for j in range(2):
                                dd = 2 * w + j
                                for v in range(W2):
                                    nc.tensor.matmul(
                                        qp[:, j, :],
                                        wq8[v][:, :, dd * P:(dd + 1) * P],
                                        lnp[v], perf_mode=DR,
                                        start=(v == 0), stop=(v == W2 - 1))
                            nc.scalar.activation(out=QT[tb][w], in_=qp,
                                                 func=FP.Copy)

                for step in range(NTB + 2):
                    if step < NTB:
                        b1_block(step)
                    if 1 <= step < NTB + 1:
                        z_bc(step - 1)
                        z_block(step - 1)
                    if step >= 2:
                        b2_block(step - 2)

            # =========== Phase C: attention ===========
            with (
                tc.tile_pool(name="es", bufs=6) as es,
                tc.tile_pool(name="outts", bufs=2) as outts,
                tc.tile_pool(name="dens", bufs=2) as dens,
                tc.tile_pool(name="fins", bufs=2) as fins,
                tc.tile_pool(name="xrs", bufs=2) as xrs,
                tc.tile_pool(name="ps_s", bufs=3, space="PSUM") as ps_s,
                tc.tile_pool(name="ps_o", bufs=1, space="PSUM") as ps_o,
                tc.tile_pool(name="ps_d", bufs=1, space="PSUM") as ps_d,
            ):
                def make_tail(qb, outTp, den_ps, xr, last=False):
                    def tail():
                        qs = slice(qb * TBLK, (qb + 1) * TBLK)
                        den_row = dens.tile([1, TBLK], F32, tag="den_row",
                                            name=f"den_row{qb}")
                        nc.scalar.activation(out=den_row, in_=den_ps, func=FP.Copy)
                        # [1,512] -> [128,4] partition-major via 4 PE transposes
                        # (a DRAM roundtrip here costs ~5us of dead latency on
                        # the final tail)
                        den_pm = ps_d.tile([P, CB], F32, tag="pd",
                                           name=f"den_pm{qb}")
                        for qq in range(CB):
                            nc.tensor.transpose(den_pm[:, qq:qq + 1],
                                                den_row[:, qq * P:(qq + 1) * P],
                                                ones11)
                        recT = dens.tile([P, CB], F32, tag="recT", name=f"recT{qb}")
                        nc.vector.reciprocal(out=recT, in_=den_pm)
                        finb = fins.tile([P, CB, C], F32, tag="finb", name=f"finb{qb}")
                        # proj (fp8 DR) + normalize + residual per 128-query slice
                        for qq in range(CB):
                            # the last tail has no kt-loop to overlap with; the
                            # pscr banks are free then, so rotate pf through
                            # them to break the pf->fin->pf serial chain
                            if last:
                                pf = ps_s.tile([P, C], F32, tag="pscr",
                                               name=f"pf{qb}_{qq}")
                            else:
                                pf = ps_d.tile([P, C], F32, tag="pd",
                                               name=f"pf{qb}_{qq}")
                            for w in range(W2):
                                nc.tensor.matmul(
                                    pf, outTp[w][:, :, qq * P:(qq + 1) * P],
                                    wp8[w], perf_mode=DR,
                                    start=(w == 0), stop=(w == W2 - 1))
                            nc.vector.scalar_tensor_tensor(
                                out=finb[:, qq, :], in0=pf,
                                scalar=recT[:, qq:qq + 1], in1=xr[:, qq, :],
                                op0=ALU.mult, op1=ALU.add)
                            rsl = slice(qb * TBLK + qq * P, qb * TBLK + (qq + 1) * P)
                            nc.gpsimd.dma_start(out=out[rsl, :], in_=finb[:, qq, :])
                    return tail

                pending_tail = None
                for qb in range(NQB):
                    qs = slice(qb * TBLK, (qb + 1) * TBLK)
                    # prefetch residual rows for this qb's tail
                    xr = xrs.tile([P, CB, C], F32, tag="xr", name=f"xr{qb}")
                    nc.gpsimd.dma_start(
                        out=xr,
                        in_=xres[qs, :].rearrange("(qq p) c -> p qq c", qq=CB))
                    pop = [ps_o.tile([P, 2, TBLK], F32, tag=f"po{w}",
                                     name=f"po{qb}_{w}") for w in range(W2)]
                    dacc = dens.tile([P, 2, TBLK], F32, tag="dacc", name=f"dacc{qb}")
                    dacc_bf = dens.tile([P, 2, TBLK], BF16, tag="dacc_bf",
                                        name=f"dacc_bf{qb}")

                    pair_t = {}

                    def scores_exp(kt, qb=qb):
                        u = kt // 2
                        if kt % 2 == 0:
                            pair_t[u] = es.tile([P, 2, TBLK], FP8, tag="e",
                                                name=f"e{qb}_{u}")
                        ksl = slice(kt * P, (kt + 1) * P)
                        pscr = ps_s.tile([P, TBLK], F32, tag="pscr",
                                         name=f"pscr{qb}_{kt}")
                        for w in range(W2):
                            nc.tensor.matmul(pscr, KT[w][:, :, ksl], QT[qb][w],
                                             perf_mode=DR,
                                             start=(w == 0), stop=(w == W2 - 1))
                        # shifted exp (softmax-invariant) keeps E in fp8e4m3 range
                        nc.scalar.activation(out=pair_t[u][:, kt % 2, :], in_=pscr,
                                             func=FP.Exp, scale=SCALE / (WS * WS),
                                             bias=neg2)

                    scores_exp(0)
                    scores_exp(1)
                    for kt in range(NKT):
                        u = kt // 2
                        if kt + 2 < NKT:
                            scores_exp(kt + 2)
                        if kt % 2 == 1:
                            for cc in range(CB):
                                nc.tensor.matmul(
                                    pop[cc // 2][:, cc % 2, :],
                                    V[u][:, :, cc * P:(cc + 1) * P], pair_t[u],
                                    perf_mode=DR,
                                    start=(u == 0), stop=(u == NKT // 2 - 1))
                            # denominator partial sums on the (idle) DVE; the
                            # last pair skips the accumulator (it feeds the
                            # PSUM reduce directly via a DR ones-matmul) so the
                            # qb-end chain doesn't wait an add+cast after the
                            # final exp
                            if u == 0:
                                nc.vector.tensor_copy(out=dacc, in_=pair_t[u])
                            elif u < NKT // 2 - 1:
                                nc.vector.tensor_add(out=dacc, in0=dacc,
                                                     in1=pair_t[u])
                                if u == NKT // 2 - 2:
                                    nc.vector.tensor_copy(out=dacc_bf, in_=dacc)
                        if kt == 6 and pending_tail is not None:
                            pending_tail()
                            pending_tail = None
                    # partition-reduce: dacc_bf (u0..14, cast early) + the last
                    # exp pair directly (den's PSUM bank stays free until here
                    # so the next qb's tail, which shares it, never waits)
                    den_ps = ps_d.tile([1, TBLK], F32, tag="pd", name=f"den{qb}")
                    for j in range(2):
                        nc.tensor.matmul(den_ps, ones_col_bf, dacc_bf[:, j, :],
                                         start=(j == 0), stop=False)
                    nc.tensor.matmul(den_ps, ones_pair8, pair_t[NKT // 2 - 1],
                                     perf_mode=DR, start=False, stop=True)
                    # evict numerators to fp8 (scaled by 1/WS^2: pf comes out as
                    # num*wp, normalized by 1/den at the fin STT)
                    outTp = []
                    for w in range(W2):
                        t = outts.tile([P, 2, TBLK], FP8, tag=f"outT{w}",
                                       name=f"outT{qb}_{w}")
                        if w == 0 or qb == NQB - 1:
                            nc.scalar.activation(out=t, in_=pop[w], func=FP.Copy,
                                                 scale=1.0 / (WS * WS))
                        else:
                            nc.vector.tensor_scalar_mul(t, pop[w], 1.0 / (WS * WS))
                        outTp.append(t)
                    pending_tail = make_tail(qb, outTp, den_ps, xr,
                                             last=(qb == NQB - 1))
                if pending_tail is not None:
                    pending_tail()
    split_multiwaits(nc)
    return nc


_NC = {}


def kernel(x, ln_gamma, ln_beta, w_qkv, w_proj, **run_kwargs):
    import ml_dtypes
    x = np.ascontiguousarray(np.asarray(x, dtype=np.float32))
    ln_gamma = np.asarray(ln_gamma, dtype=np.float32)
    ln_beta = np.asarray(ln_beta, dtype=np.float32)
    fp8_np = mybir.dt.np(FP8)
    # channel-paired fp8 qkv weights, prescaled by WS:
    # wq8[w, p, j, d] = w_qkv[d, w*256 + j*128 + p] * WS
    wq8 = np.ascontiguousarray(
        (np.asarray(w_qkv, dtype=np.float32).T * WS)
        .reshape(W2, 2, P, 3 * C).transpose(0, 2, 1, 3)).astype(fp8_np)
    wp8 = np.ascontiguousarray(
        (np.asarray(w_proj, dtype=np.float32).T * WS)
        .reshape(W2, 2, P, C).transpose(0, 2, 1, 3)).astype(fp8_np)
    b, c, h, w = x.shape
    assert (b, c, h * w) == (4, C, T)

    in_maps = []
    for core in range(8):
        bi, half = core // 2, core % 2
        xt_b = x[bi].reshape(C, T)
        if half == 0:
            xt_i = xt_b
        else:
            xt_i = np.concatenate([xt_b[:, TQ:], xt_b[:, :TQ]], axis=1)
        xt_i = np.ascontiguousarray(xt_i)
        xres_i = np.ascontiguousarray(xt_i[:, :TQ].T)
        in_maps.append({
            "xbf": xt_i.astype(ml_dtypes.bfloat16),
            "xres": xres_i, "wq8d": wq8, "wp8d": wp8,
            "gamma": ln_gamma, "beta": ln_beta,
        })

    # specialized plain-gamma/beta build measured slower (DVE became the
    # phase-B cap once the fp8 LN writes moved there); keep the general path
    plain_gb = False
    if plain_gb not in _NC:
        _NC[plain_gb] = build_nc(plain_gb=plain_gb)
    res = run_bass_kernel_spmd(_NC[plain_gb], in_maps, core_ids=list(range(8)),
                               **run_kwargs)

    y = np.empty((b, T, C), dtype=np.float32)
    for core in range(8):
        bi, half = core // 2, core % 2
        y[bi, half * TQ:(half + 1) * TQ, :] = res.results[core]["out"]
    y = np.ascontiguousarray(y.transpose(0, 2, 1).reshape(b, C, h, w))
    if run_kwargs:
        return y, res
    return y
